# revision 35
# baseline (speedup 1.0000x reference)
"""ObjectDecoder kernel for Trainium2 (8 NeuronCores, data-parallel over batch).

Computes out[b, o, a, p, k] = sum_d x[b, o, d] * W[o, a, p, d, k] + bias[o, a, p, k]
  x: [16384, 16, 256] f32, W: [16, 4, 2, 256, 8] f32, b: [16, 4, 2, 8] f32
  out: [16384, 16, 4, 2, 8] f32

Per-core plan (batch shard of 2048 rows).  x ships as FP8 E3M4 (float8e3):
E3M4 holds N(0,1) x with ~2^-5 relative error and the PE consumes it
DIRECTLY in a mixed-dtype matmul against bf16 weights (verified bit-exact on
HW), so the x stream is 8.4 MB instead of bf16's 16.8 MB and there is no
on-chip cast.  End-to-end rel err 1.64e-2 vs the 2e-2 gate, deterministic
(seed-0 inputs; the only approximations are host-side rounding plus the
int8 output quantize, all independent of execution order).

The PE matmul stream (128 x [K=128, M=64, N=512] f8e3 at ~215 ns) is ~27.5
us and is the critical path; around it sit ~6 us of fixed framework
preamble, ~2 us of fill, ~2.5 us of drain and ~2 us of semaphore sweep.
Hard-won scheduling details:

  - Streaming granularity: one dma_start per object ([128, 2, BS], 4 KiB
    partition lines -> ~4 KiB descriptors spread over all 16 DMA queues,
    the measured throughput sweet spot: the stream runs at the ~340 B/ns
    per-core read cap).  Whole-pair dmas lower to one 8 KiB descriptor per
    partition and the queues idle ~40% between descriptors; k-half dmas
    double the dispatch count and slow the stream ~20%; batch-sliced dmas
    shred into ~100 B descriptors and are 10x slower.  W is laid out
    partition-outermost ([128, pair, k, o2, apk]) and moves as just two
    dmas: the pair-0 slice first (so the first matmuls are not gated on
    the full 0.5 MB of W), then the rest.
  - Ring use: the sync ring carries W + pairs 0-6 (first byte ~1.5 us after
    dispatch).  The scalar ring (~4.3 us to first byte) carries bias +
    pair 7 + all output stores; pair 7 is resident ~15 us before the PE
    needs it so the stream tail never stalls the PE.  All 16 object tiles
    stay resident in SBUF (64 KiB/partition) - no pool recycling stalls.
  - PSUM: all 8 banks rotate through one pool (the warm-up bank is recycled
    into the rotation).  Evacuation: scalar-engine activation fuses
    (psum + b) * OSCALE with the int8 quantize; ONE whole-pair store per
    pair issued from the same engine (same-engine ordering keeps ACT
    writes DMA-visible without cross-engine races; per-chunk stores crowd
    the sequencer with ~0.6 us DIRECT2D dispatches and back up the drain).
    The last pair stores in two halves so the final transfer is short.
  - The first three pairs trail the stream by less than one object, so
    their matmuls run o2-outer (all of object 0, then object 1) to absorb
    the ~1.5 us arrival skew; later pairs run o2-inner so consecutive
    matmuls alternate PE column strips 0/64 and LDWEIGHTS overlaps the
    in-flight MATMUL.
  - 4 warm-up matmuls on a zeroed tile while W/x are in flight release the
    PE HAM clock gate (1.2 -> 2.4 GHz) before real matmuls start.
  - PE pair order [0..5, 7, 6]: pair 7 (scalar ring, resident early) is
    processed before pair 6, whose x is the sync stream's final transfer -
    trace-verified to cut the post-stream matmul tail from ~4.4 to ~2.3 us.

Measured: 45.8-50.7 us across runs of this exact config (device thermal
state: on slow runs the PE per-matmul time and the DMA stream rate degrade
together - P0 downclock), vs 70.6 us for the bf16 version.  Remaining fixed costs: ~6 us framework preamble, ~2.2 us
HWDGE ring-start latency + ~2 us fill, ~27.5 us PE stream (the floor:
1 moving elem/cycle/partition at 2.4 GHz), ~3 us drain, ~2.5 us semaphore
sweep.
"""

import os
from contextlib import ExitStack

os.environ.setdefault("JAX_PLATFORMS", "axon")

import numpy as np
import ml_dtypes

import concourse.bass as bass
import concourse.mybir as mybir
import concourse.tile as tile
from concourse import bacc
from concourse.bass_utils import run_bass_kernel_spmd

B, N_OBJ, DIM_IN, APK = 16384, 16, 256, 64
N_CORES = 8
BS = B // N_CORES          # 2048 batch rows per core
NT = 512                   # moving-operand tile (one PSUM bank of fp32)
NB = BS // NT              # 4 batch chunks per core
F32 = mybir.dt.float32
BF16 = mybir.dt.bfloat16
F8E3 = mybir.dt.float8e3
I8 = mybir.dt.int8
NP_BF16 = ml_dtypes.bfloat16
NP_E3M4 = ml_dtypes.float8_e3m4
# Output quantized to int8 (out = (psum + b) * OSCALE, decoded on host by
# /OSCALE). |out| <= ~3.39, range +-4 -> step ~0.031.
OSCALE = 127.0 / 4.0

_CACHE: dict = {}


def _build_nc(variant=None):
    if variant is None:
        variant = os.environ.get("KVARIANT", "v6")
    n_warm = int(os.environ.get("WARMUP_MMS", "4"))
    nc = bacc.Bacc(
        "TRN2",
        target_bir_lowering=False,
        debug=False,
        enable_partition_id=False,
    )

    # xt[o, p, k, b]: d = k*128 + p - 4 KiB contiguous per partition line
    xt = nc.declare_dram_parameter("xt", [N_OBJ, 128, 2, BS], F8E3, isOutput=False)
    # wt[p, pair, k, o2, apk]: partition axis outermost so per-pair slices
    # stay legal dmas with contiguous per-partition runs
    wt = nc.declare_dram_parameter(
        "wt", [128, N_OBJ // 2, 2, 2, APK], BF16, isOutput=False
    )
    bt = nc.declare_dram_parameter("bt", [128, N_OBJ // 2], F32, isOutput=False)
    out = nc.declare_dram_parameter("out", [N_OBJ, APK, BS], I8, isOutput=True)

    n_pairs = N_OBJ // 2

    with tile.TileContext(nc) as tc, ExitStack() as ctx:
        wpool = ctx.enter_context(tc.tile_pool(name="w", bufs=1))
        # bufs is per unique tile name: 16 uniquely-named tiles, 1 buf each
        # = every object resident in SBUF for the whole kernel (64 KiB/part)
        xpool = ctx.enter_context(tc.tile_pool(name="x", bufs=1))
        psum = ctx.enter_context(
            tc.tile_pool(name="ps", bufs=8, space=bass.MemorySpace.PSUM)
        )
        opool = ctx.enter_context(tc.tile_pool(name="o", bufs=8))

        w_sb = wpool.tile([128, n_pairs, 2, 2, APK], BF16)
        b_sb = wpool.tile([128, n_pairs], F32)
        xts = {}
        for o in range(N_OBJ):
            xts[o] = xpool.tile([128, 2, BS], F8E3, name=f"xo{o}")

        # whole-object x dmas: 4 KiB partition lines lower to ~4 KiB
        # descriptors, the measured throughput sweet spot (2 KiB halves ran
        # 22% slower end-to-end, 8 KiB whole-pairs 35% slower).  W moves as
        # just two dmas (pair-0 slice, then the rest) - per-pair W slices
        # cost 7 extra ~0.5 us dispatch slots on the sync ring and delayed
        # the whole x stream.
        # sync ring: W-pair0, pair-0 x, W-rest, pairs 1-6 x.
        # scalar ring: bias + pair 7 early (resident long before the PE
        # reaches it), then the stores.
        nc.sync.dma_start(w_sb[:, 0:1], wt[:, 0:1])
        nc.sync.dma_start(xts[0][:], xt[0])
        nc.sync.dma_start(xts[1][:], xt[1])
        nc.sync.dma_start(w_sb[:, 1:n_pairs], wt[:, 1:n_pairs])
        nc.scalar.dma_start(b_sb[:], bt[:])

        # PE warm-up: matmuls on a zeroed tile while W/x stream in, so
        # the HAM clock gate releases (1.2 -> 2.4 GHz) before real matmuls
        # start.  The warm bank is named into the main pool's rotation.
        if n_warm:
            junk = wpool.tile([128, NT + 64], BF16)
            nc.vector.memset(junk[:], 0)
            junk_ps = psum.tile([128, NT], F32, name="ps")
            for _ in range(n_warm):
                nc.tensor.matmul(
                    junk_ps[0:64, :],
                    junk[:, NT : NT + 64],
                    junk[:, :NT],
                    start=True,
                    stop=True,
                )

        # tail-only reorder: pair 7 (scalar ring, SBUF-resident since
        # ~14 us) is processed BEFORE pair 6, whose x is the sync stream's
        # final transfer - so only one pair of matmuls (~3.5 us) remains
        # after the last x byte lands instead of two
        pe_order = [0, 1, 2, 3, 4, 5, n_pairs - 1, n_pairs - 2]
        for pi, op in enumerate(pe_order):  # object pairs
            # each pair's object 0 on the sync ring, object 1 on the scalar
            # ring: one ring alone sustains only ~260 B/ns (descriptor
            # processing), well under the ~340 B/ns fabric cap, and the
            # scalar ring is otherwise idle mid-kernel - splitting brings
            # pair cadence under the PE's 3.45 us consumption rate
            if 1 <= op <= n_pairs - 2:
                nc.sync.dma_start(xts[2 * op][:], xt[2 * op])
                nc.scalar.dma_start(xts[2 * op + 1][:], xt[2 * op + 1])
                if op == 5:
                    # pair 7's x must be dispatched before its matmuls,
                    # which run at position 6 (before pair 6)
                    nc.scalar.dma_start(xts[14][:], xt[14])
                    nc.scalar.dma_start(xts[15][:], xt[15])

            ot = opool.tile([128, BS], I8)
            pss = [psum.tile([128, NT], F32, name="ps") for n in range(NB)]
            if pi <= 2:
                # early pairs trail the stream by less than one object, and
                # object 1 lands ~1.5 us after object 0: run all of object
                # 0's matmuls first (costs some LDWEIGHTS overlap, saves
                # the arrival stall)
                mm_order = [
                    (n, k, o2) for o2 in range(2) for n in range(NB) for k in range(2)
                ]
            else:
                # o2 innermost alternates PE column strips 0/64 so
                # LDWEIGHTS overlaps the in-flight MATMUL
                mm_order = [
                    (n, k, o2) for n in range(NB) for k in range(2) for o2 in range(2)
                ]
            for n, k, o2 in mm_order:
                nc.tensor.matmul(
                    pss[n][o2 * 64 : (o2 + 1) * 64, :],
                    w_sb[:, op, k, o2, :],
                    xts[2 * op + o2][:, k, n * NT : (n + 1) * NT],
                    start=(k == 0),
                    stop=(k == 1),
                )
            fine = pi == n_pairs - 1
            for n in range(NB):
                # fused quantizing evacuation: int8((psum + b) * OSCALE);
                # bt already holds b * OSCALE (host pre-scaled)
                nc.scalar.activation(
                    ot[:, n * NT : (n + 1) * NT],
                    pss[n][:],
                    mybir.ActivationFunctionType.Identity,
                    bias=b_sb[:, op : op + 1],
                    scale=OSCALE,
                )
                if fine and n == 1:
                    # first half of the last pair's store overlaps the
                    # remaining evacs; the post-evac transfer is short
                    nc.scalar.dma_start(
                        out[2 * op : 2 * op + 2, :, : 2 * NT], ot[:, : 2 * NT]
                    )
            if fine:
                nc.scalar.dma_start(
                    out[2 * op : 2 * op + 2, :, 2 * NT :], ot[:, 2 * NT :]
                )
            else:
                nc.scalar.dma_start(out[2 * op : 2 * op + 2, :, :], ot[:])

    nc.compile()
    return nc


def _get_nc():
    if "nc" not in _CACHE:
        _CACHE["nc"] = _build_nc()
    return _CACHE["nc"]


def _prep_inputs(x, W, b):
    # x f32 -> fp8 E3M4 bytes (the device reads them as float8e3 directly)
    x8 = np.asarray(x, dtype=np.float32).astype(NP_E3M4).view(np.uint8)
    # wt[p, pair, k, o2, apk] from W[o,a,p,d,k]: d = k*128 + p, o = 2*pair+o2
    wt = np.ascontiguousarray(
        np.asarray(W, dtype=np.float32)
        .astype(NP_BF16)
        .transpose(3, 0, 1, 2, 4)          # [d, o, a, p, k]
        .reshape(2, 128, N_OBJ, APK)       # [k, p, o, apk]
        .transpose(1, 2, 0, 3)             # [p, o, k, apk]
        .reshape(128, N_OBJ // 2, 2, 2, APK)   # [p, pair, o2, k, apk]
        .transpose(0, 1, 3, 2, 4)          # [p, pair, k, o2, apk]
    )
    # bt[o2*64+apk, pair] - fp32, pre-scaled by OSCALE for the int8-quantizing
    # activation (out = psum*OSCALE + b*OSCALE)
    bt = np.ascontiguousarray(
        (np.asarray(b, dtype=np.float32) * OSCALE)
        .reshape(N_OBJ // 2, 2, APK)
        .transpose(1, 2, 0)
        .reshape(128, N_OBJ // 2)
    )
    in_maps = []
    for c in range(N_CORES):
        xs = x8[c * BS : (c + 1) * BS]  # [BS, 16, 256] uint8 (e3m4 bytes)
        # xt[o, p, k, b] with d = k*128 + p (4 KiB contiguous per (o, p))
        xtc = np.ascontiguousarray(
            xs.transpose(1, 2, 0).reshape(N_OBJ, 2, 128, BS).transpose(0, 2, 1, 3)
        )
        in_maps.append({"xt": xtc, "wt": wt, "bt": bt})
    return in_maps


def kernel(x, W, b, _trace=False, **run_kwargs):
    nc = _get_nc()
    in_maps = _prep_inputs(x, W, b)
    res = run_bass_kernel_spmd(
        nc, in_maps, core_ids=list(range(N_CORES)), trace=_trace, **run_kwargs
    )
    _CACHE["last_results"] = res
    out = np.empty((B, N_OBJ, APK), dtype=np.float32)
    inv = np.float32(1.0 / OSCALE)
    for c in range(N_CORES):
        # out_t[o, apk, batch] -> [batch, o, apk]; decode int8 -> f32
        out[c * BS : (c + 1) * BS] = (
            res.results[c]["out"].astype(np.float32) * inv
        ).transpose(2, 0, 1)
    return out.reshape(B, N_OBJ, 4, 2, 8)


# revision 36
# speedup vs baseline: 1.2301x; 1.2301x over previous
"""ObjectDecoder kernel for Trainium2 (8 NeuronCores, data-parallel over batch).

Computes out[b, o, a, p, k] = sum_d x[b, o, d] * W[o, a, p, d, k] + bias[o, a, p, k]
  x: [16384, 16, 256] f32, W: [16, 4, 2, 256, 8] f32, b: [16, 4, 2, 8] f32
  out: [16384, 16, 4, 2, 8] f32

Per-core plan (batch shard of 2048 rows).  x ships as FP8 E3M4 (float8e3):
E3M4 holds N(0,1) x with ~2^-5 relative error and the PE consumes it
DIRECTLY in a mixed-dtype matmul against bf16 weights (verified bit-exact on
HW), so the x stream is 8.4 MB instead of bf16's 16.8 MB and there is no
on-chip cast.  End-to-end rel err 1.64e-2 vs the 2e-2 gate, deterministic
(seed-0 inputs; the only approximations are host-side rounding plus the
int8 output quantize, all independent of execution order).

The PE matmul stream (128 x [K=128, M=64, N=512] f8e3 at ~215 ns) is ~27.5
us and is the critical path; around it sit ~6 us of fixed framework
preamble, ~2 us of fill, ~2.5 us of drain and ~2 us of semaphore sweep.
Hard-won scheduling details:

  - Streaming granularity: one dma_start per object ([128, 2, BS], 4 KiB
    partition lines -> ~4 KiB descriptors spread over all 16 DMA queues,
    the measured throughput sweet spot: the stream runs at the ~340 B/ns
    per-core read cap).  Whole-pair dmas lower to one 8 KiB descriptor per
    partition and the queues idle ~40% between descriptors; k-half dmas
    double the dispatch count and slow the stream ~20%; batch-sliced dmas
    shred into ~100 B descriptors and are 10x slower.  W is laid out
    partition-outermost ([128, pair, k, o2, apk]) and moves as just two
    dmas: the pair-0 slice first (so the first matmuls are not gated on
    the full 0.5 MB of W), then the rest.
  - Ring use: the sync ring carries W + pairs 0-6 (first byte ~1.5 us after
    dispatch).  The scalar ring (~4.3 us to first byte) carries bias +
    pair 7 + all output stores; pair 7 is resident ~15 us before the PE
    needs it so the stream tail never stalls the PE.  All 16 object tiles
    stay resident in SBUF (64 KiB/partition) - no pool recycling stalls.
  - PSUM: all 8 banks rotate through one pool (the warm-up bank is recycled
    into the rotation).  Evacuation: scalar-engine activation fuses
    (psum + b) * OSCALE with the int8 quantize; ONE whole-pair store per
    pair issued from the same engine (same-engine ordering keeps ACT
    writes DMA-visible without cross-engine races; per-chunk stores crowd
    the sequencer with ~0.6 us DIRECT2D dispatches and back up the drain).
    The last pair stores in two halves so the final transfer is short.
  - The first three pairs trail the stream by less than one object, so
    their matmuls run o2-outer (all of object 0, then object 1) to absorb
    the ~1.5 us arrival skew; later pairs run o2-inner so consecutive
    matmuls alternate PE column strips 0/64 and LDWEIGHTS overlaps the
    in-flight MATMUL.
  - 4 warm-up matmuls on a zeroed tile while W/x are in flight release the
    PE HAM clock gate (1.2 -> 2.4 GHz) before real matmuls start.
  - PE pair order [0..5, 7, 6]: pair 7 (scalar ring, resident early) is
    processed before pair 6, whose x is the sync stream's final transfer -
    trace-verified to cut the post-stream matmul tail from ~4.4 to ~2.3 us.

Measured: 45.8-50.7 us across runs of this exact config (device thermal
state: on slow runs the PE per-matmul time and the DMA stream rate degrade
together - P0 downclock), vs 70.6 us for the bf16 version.  Remaining fixed costs: ~6 us framework preamble, ~2.2 us
HWDGE ring-start latency + ~2 us fill, ~27.5 us PE stream (the floor:
1 moving elem/cycle/partition at 2.4 GHz), ~3 us drain, ~2.5 us semaphore
sweep.
"""

import os
from contextlib import ExitStack

os.environ.setdefault("JAX_PLATFORMS", "axon")

import numpy as np
import ml_dtypes

import concourse.bass as bass
import concourse.mybir as mybir
import concourse.tile as tile
from concourse import bacc
from concourse.bass_utils import run_bass_kernel_spmd

B, N_OBJ, DIM_IN, APK = 16384, 16, 256, 64
N_CORES = 8
BS = B // N_CORES          # 2048 batch rows per core
NT = 512                   # moving-operand tile (one PSUM bank of fp32)
NB = BS // NT              # 4 batch chunks per core
F32 = mybir.dt.float32
BF16 = mybir.dt.bfloat16
F8E3 = mybir.dt.float8e3
I8 = mybir.dt.int8
NP_BF16 = ml_dtypes.bfloat16
NP_E3M4 = ml_dtypes.float8_e3m4
# Output quantized to int8 (out = (psum + b) * OSCALE, decoded on host by
# /OSCALE). |out| <= ~3.39, range +-4 -> step ~0.031.
OSCALE = 127.0 / 4.0

_CACHE: dict = {}


def _build_nc(variant=None):
    if variant is None:
        variant = os.environ.get("KVARIANT", "v6")
    n_warm = int(os.environ.get("WARMUP_MMS", "4"))
    nc = bacc.Bacc(
        "TRN2",
        target_bir_lowering=False,
        debug=False,
        enable_partition_id=False,
    )

    # xt[o, p, k, b]: d = k*128 + p - 4 KiB contiguous per partition line
    xt = nc.declare_dram_parameter("xt", [N_OBJ, 128, 2, BS], F8E3, isOutput=False)
    # wt[p, pair, k, o2, apk]: partition axis outermost so per-pair slices
    # stay legal dmas with contiguous per-partition runs
    wt = nc.declare_dram_parameter(
        "wt", [128, N_OBJ // 2, 2, 2, APK], BF16, isOutput=False
    )
    bt = nc.declare_dram_parameter("bt", [128, N_OBJ // 2], F32, isOutput=False)
    out = nc.declare_dram_parameter("out", [N_OBJ, APK, BS], I8, isOutput=True)

    n_pairs = N_OBJ // 2

    with tile.TileContext(nc) as tc, ExitStack() as ctx:
        wpool = ctx.enter_context(tc.tile_pool(name="w", bufs=1))
        # bufs is per unique tile name: 16 uniquely-named tiles, 1 buf each
        # = every object resident in SBUF for the whole kernel (64 KiB/part)
        xpool = ctx.enter_context(tc.tile_pool(name="x", bufs=1))
        psum = ctx.enter_context(
            tc.tile_pool(name="ps", bufs=8, space=bass.MemorySpace.PSUM)
        )
        opool = ctx.enter_context(tc.tile_pool(name="o", bufs=3))

        w_sb = wpool.tile([128, n_pairs, 2, 2, APK], BF16)
        b_sb = wpool.tile([128, n_pairs], F32)
        xts = {}
        for o in range(N_OBJ):
            xts[o] = xpool.tile([128, 2, BS], F8E3, name=f"xo{o}")

        # whole-object x dmas: 4 KiB partition lines lower to ~4 KiB
        # descriptors, the measured throughput sweet spot (2 KiB halves ran
        # 22% slower end-to-end, 8 KiB whole-pairs 35% slower).  W moves as
        # just two dmas (pair-0 slice, then the rest) - per-pair W slices
        # cost 7 extra ~0.5 us dispatch slots on the sync ring and delayed
        # the whole x stream.
        # sync ring: W-pair0, pair-0 x, W-rest, pairs 1-6 x.
        # scalar ring: bias + pair 7 early (resident long before the PE
        # reaches it), then the stores.
        nc.sync.dma_start(w_sb[:, 0:1], wt[:, 0:1])
        nc.sync.dma_start(xts[0][:], xt[0])
        nc.sync.dma_start(xts[1][:], xt[1])
        nc.sync.dma_start(w_sb[:, 1:n_pairs], wt[:, 1:n_pairs])
        nc.scalar.dma_start(b_sb[:], bt[:])
        nc.scalar.dma_start(xts[14][:], xt[14])
        nc.scalar.dma_start(xts[15][:], xt[15])

        # PE warm-up: matmuls on a zeroed tile while W/x stream in, so
        # the HAM clock gate releases (1.2 -> 2.4 GHz) before real matmuls
        # start.  The warm bank is named into the main pool's rotation.
        if n_warm:
            junk = wpool.tile([128, NT + 64], BF16)
            nc.vector.memset(junk[:], 0)
            junk_ps = psum.tile([128, NT], F32, name="ps")
            for _ in range(n_warm):
                nc.tensor.matmul(
                    junk_ps[0:64, :],
                    junk[:, NT : NT + 64],
                    junk[:, :NT],
                    start=True,
                    stop=True,
                )

        # tail-only reorder: pair 7 (scalar ring, SBUF-resident since
        # ~14 us) is processed BEFORE pair 6, whose x is the sync stream's
        # final transfer - so only one pair of matmuls (~3.5 us) remains
        # after the last x byte lands instead of two
        pe_order = [0, 1, 2, 3, 4, 5, n_pairs - 1, n_pairs - 2]
        for pi, op in enumerate(pe_order):  # object pairs
            if 1 <= op <= n_pairs - 2:
                nc.sync.dma_start(xts[2 * op][:], xt[2 * op])
                nc.sync.dma_start(xts[2 * op + 1][:], xt[2 * op + 1])

            ot = opool.tile([128, BS], I8)
            pss = [psum.tile([128, NT], F32, name="ps") for n in range(NB)]
            if pi <= 2:
                # early pairs trail the stream by less than one object, and
                # object 1 lands ~1.5 us after object 0: run all of object
                # 0's matmuls first (costs some LDWEIGHTS overlap, saves
                # the arrival stall)
                mm_order = [
                    (n, k, o2) for o2 in range(2) for n in range(NB) for k in range(2)
                ]
            else:
                # o2 innermost alternates PE column strips 0/64 so
                # LDWEIGHTS overlaps the in-flight MATMUL
                mm_order = [
                    (n, k, o2) for n in range(NB) for k in range(2) for o2 in range(2)
                ]
            for n, k, o2 in mm_order:
                nc.tensor.matmul(
                    pss[n][o2 * 64 : (o2 + 1) * 64, :],
                    w_sb[:, op, k, o2, :],
                    xts[2 * op + o2][:, k, n * NT : (n + 1) * NT],
                    start=(k == 0),
                    stop=(k == 1),
                )
            fine = pi == n_pairs - 1
            for n in range(NB):
                # fused quantizing evacuation: int8((psum + b) * OSCALE);
                # bt already holds b * OSCALE (host pre-scaled)
                nc.scalar.activation(
                    ot[:, n * NT : (n + 1) * NT],
                    pss[n][:],
                    mybir.ActivationFunctionType.Identity,
                    bias=b_sb[:, op : op + 1],
                    scale=OSCALE,
                )
                if fine and n == 1:
                    # first half of the last pair's store overlaps the
                    # remaining evacs; the post-evac transfer is short
                    nc.scalar.dma_start(
                        out[2 * op : 2 * op + 2, :, : 2 * NT], ot[:, : 2 * NT]
                    )
            if fine:
                nc.scalar.dma_start(
                    out[2 * op : 2 * op + 2, :, 2 * NT :], ot[:, 2 * NT :]
                )
            else:
                nc.scalar.dma_start(out[2 * op : 2 * op + 2, :, :], ot[:])

    nc.compile()
    return nc


def _get_nc():
    if "nc" not in _CACHE:
        _CACHE["nc"] = _build_nc()
    return _CACHE["nc"]


def _prep_inputs(x, W, b):
    # x f32 -> fp8 E3M4 bytes (the device reads them as float8e3 directly)
    x8 = np.asarray(x, dtype=np.float32).astype(NP_E3M4).view(np.uint8)
    # wt[p, pair, k, o2, apk] from W[o,a,p,d,k]: d = k*128 + p, o = 2*pair+o2
    wt = np.ascontiguousarray(
        np.asarray(W, dtype=np.float32)
        .astype(NP_BF16)
        .transpose(3, 0, 1, 2, 4)          # [d, o, a, p, k]
        .reshape(2, 128, N_OBJ, APK)       # [k, p, o, apk]
        .transpose(1, 2, 0, 3)             # [p, o, k, apk]
        .reshape(128, N_OBJ // 2, 2, 2, APK)   # [p, pair, o2, k, apk]
        .transpose(0, 1, 3, 2, 4)          # [p, pair, k, o2, apk]
    )
    # bt[o2*64+apk, pair] - fp32, pre-scaled by OSCALE for the int8-quantizing
    # activation (out = psum*OSCALE + b*OSCALE)
    bt = np.ascontiguousarray(
        (np.asarray(b, dtype=np.float32) * OSCALE)
        .reshape(N_OBJ // 2, 2, APK)
        .transpose(1, 2, 0)
        .reshape(128, N_OBJ // 2)
    )
    in_maps = []
    for c in range(N_CORES):
        xs = x8[c * BS : (c + 1) * BS]  # [BS, 16, 256] uint8 (e3m4 bytes)
        # xt[o, p, k, b] with d = k*128 + p (4 KiB contiguous per (o, p))
        xtc = np.ascontiguousarray(
            xs.transpose(1, 2, 0).reshape(N_OBJ, 2, 128, BS).transpose(0, 2, 1, 3)
        )
        in_maps.append({"xt": xtc, "wt": wt, "bt": bt})
    return in_maps


def kernel(x, W, b, _trace=False, **run_kwargs):
    nc = _get_nc()
    in_maps = _prep_inputs(x, W, b)
    res = run_bass_kernel_spmd(
        nc, in_maps, core_ids=list(range(N_CORES)), trace=_trace, **run_kwargs
    )
    _CACHE["last_results"] = res
    out = np.empty((B, N_OBJ, APK), dtype=np.float32)
    inv = np.float32(1.0 / OSCALE)
    for c in range(N_CORES):
        # out_t[o, apk, batch] -> [batch, o, apk]; decode int8 -> f32
        out[c * BS : (c + 1) * BS] = (
            res.results[c]["out"].astype(np.float32) * inv
        ).transpose(2, 0, 1)
    return out.reshape(B, N_OBJ, 4, 2, 8)
